# revision 1
# baseline (speedup 1.0000x reference)
"""Multi-head attention (B=2, N=2048, C=1024, H=16, D=64) on 8 TRN2 NeuronCores.

Sharding: core c = (batch b = c//4) x (head-group g = c%4 -> heads 4g..4g+3).
Data parallel on B, tensor parallel on heads; fp16 ReduceScatter of the
out-projection partials within each 4-core batch group.

Everything on device stays transposed ([channel, position]); the host
pre-transposes inputs and post-transposes the output.
"""

import numpy as np

import concourse.bacc as bacc
import concourse.tile as tile
import concourse.mybir as mybir
from concourse.bass_utils import run_bass_kernel_spmd

B, N, C, H = 2, 2048, 1024, 16
D = C // H          # 64
HL = H // 4         # 4 heads per core
CL = HL * D         # 256 local channels
N_CORES = 8
GROUPS = [[0, 1, 2, 3], [4, 5, 6, 7]]

F32 = mybir.dt.float32
BF16 = mybir.dt.float16
BF = np.float16

KC = C // 128       # 8  K-chunks of the input channel dim
NI = N // 512       # 4  512-wide i-chunks
NJ = N // 128       # 16 128-row j-chunks


def build_kernel(n_cores=N_CORES, groups=GROUPS):
    group_size = len(groups[0])
    rs_out_rows = C // group_size

    nc = bacc.Bacc("TRN2", target_bir_lowering=False, debug=False,
                   num_devices=n_cores)

    xT = nc.declare_dram_parameter("xT", [C, N], BF16, isOutput=False)
    cos2 = nc.declare_dram_parameter("cos2", [128, N], BF16, isOutput=False)
    sin2s = nc.declare_dram_parameter("sin2s", [128, N], BF16, isOutput=False)
    wqkT = nc.declare_dram_parameter("wqkT", [C, 2 * CL], BF16, isOutput=False)
    bqk = nc.declare_dram_parameter("bqk", [2 * CL, 1], F32, isOutput=False)
    wvT = nc.declare_dram_parameter("wvT", [C, CL], BF16, isOutput=False)
    wprojT = nc.declare_dram_parameter("wprojT", [CL, C], BF16, isOutput=False)
    beff = nc.declare_dram_parameter("beff", [rs_out_rows, 1], F32, isOutput=False)
    out = nc.declare_dram_parameter("out", [rs_out_rows, N], F32, isOutput=True)

    with tile.TileContext(nc) as tc:
        with tc.tile_pool(name="dram", bufs=1, space="DRAM") as dram, \
             tc.tile_pool(name="sbuf", bufs=1) as sb, \
             tc.tile_pool(name="psum", bufs=1, space="PSUM") as ps:

            # tile for clock-warming matmuls (see _warm_pe)
            warm = sb.tile([128, 128], BF16, name="warm", tag="warm")
            nc.vector.memset(warm[:], 0.001)

            def _warm_pe(tag, n):
                # short matmuls alternating two PSUM tiles: keeps the PE's
                # activity monitor busy so the clock gate stays at full rate
                wps = [ps.tile([128, 64], F32, name=f"warmp{tag}_{a}",
                               tag="sc", bufs=2) for a in range(2)]
                for r in range(n):
                    nc.tensor.matmul(wps[r % 2][:], warm[:], warm[:, :64],
                                     start=True, stop=True)

            # run a warm burst during the input-DMA dead window so the qk
            # projection starts with the clock gate already released
            _warm_pe("s", 24)

            # ---- load inputs (wqk/xb interleaved so the qk matmuls can start
            # before the full x transfer lands) ----
            # both HWDGE queues (sync + scalar) share the bulk input load
            xb, wqk_sb = [], []
            for kc in range(KC):
                t = sb.tile([128, 2 * CL], BF16, name=f"wqk{kc}", tag=f"wqk{kc}")
                eng = nc.scalar if kc % 2 == 0 else nc.sync
                eng.dma_start(t[:], wqkT.ap()[128 * kc:128 * (kc + 1), :])
                wqk_sb.append(t)
                t = sb.tile([128, N], BF16, name=f"xb{kc}", tag=f"xb{kc}")
                eng = nc.sync if kc % 2 == 0 else nc.scalar
                eng.dma_start(t[:], xT.ap()[128 * kc:128 * (kc + 1), :])
                xb.append(t)
            wv_sb = []
            for kc in range(KC):
                t = sb.tile([128, CL], BF16, name=f"wv{kc}", tag=f"wv{kc}")
                nc.sync.dma_start(t[:], wvT.ap()[128 * kc:128 * (kc + 1), :])
                wv_sb.append(t)
            cos_sb = sb.tile([128, N], BF16, name="cos_sb", tag="cos_sb")
            nc.sync.dma_start(cos_sb[:], cos2.ap())
            sin_sb = sb.tile([128, N], BF16, name="sin_sb", tag="sin_sb")
            nc.scalar.dma_start(sin_sb[:], sin2s.ap())
            bqk_sb = []
            for m in range(4):
                t = sb.tile([128, 1], F32, name=f"bqk{m}", tag=f"bqk{m}")
                nc.sync.dma_start(t[:], bqk.ap()[128 * m:128 * (m + 1), :])
                bqk_sb.append(t)
            wproj_sb = []
            for p in range(2):
                t = sb.tile([128, C], BF16, name=f"wproj{p}", tag=f"wproj{p}")
                nc.sync.dma_start(t[:], wprojT.ap()[128 * p:128 * (p + 1), :])
                wproj_sb.append(t)
            beff_sb = []
            for m in range(rs_out_rows // 128):
                t = sb.tile([128, 1], F32, name=f"beff{m}", tag=f"beff{m}")
                nc.sync.dma_start(t[:], beff.ap()[128 * m:128 * (m + 1), :])
                beff_sb.append(t)

            # ---- qk projection + RoPE ----
            # chunk m rows: m=0:[q_h0,q_h1] m=1:[q_h2,q_h3] m=2:[k_h0,k_h1] m=3:[k_h2,k_h3]
            # so q and k of head h sit at the same partition offset 64*(h%2).
            # k of each head lands in its own zero-padded [128, N] tile so the
            # scores matmul can contract over K=128 (16-bit matmuls run at
            # half rate for K=64 -- zero rows buy back the full rate).
            q_r = []      # 2 tiles: [q_h0,q_h1], [q_h2,q_h3]
            k_t = []      # 4 tiles: k_h at rows 64*(h%2), zeros elsewhere
            for h in range(4):
                kt = sb.tile([128, N], BF16, name=f"ktile{h}", tag=f"ktile{h}")
                z = slice(0, 64) if h % 2 == 1 else slice(64, 128)
                nc.vector.memset(kt[z, :], 0.0)
                k_t.append(kt)
            swap_mask = [i ^ 1 for i in range(32)]
            # kc-outer accumulation so the first matmul only needs the first
            # x/w chunk off DMA; 2 PSUM tiles hold the 4 m-accumulators
            qks_t = [sb.tile([128, N], BF16, name=f"qks{m}", tag=f"qks{m}")
                     for m in range(4)]
            for n in range(NI):
                accs = [ps.tile([128, 1024], F32, name=f"qacc{n}_{a}",
                                tag="sc", bufs=2) for a in range(2)]
                for kc in range(KC):
                    for m in range(4):
                        nc.tensor.matmul(
                            accs[m // 2][:, 512 * (m % 2):512 * (m % 2 + 1)],
                            wqk_sb[kc][:, 128 * m:128 * (m + 1)],
                            xb[kc][:, 512 * n:512 * (n + 1)],
                            start=(kc == 0), stop=(kc == KC - 1))
                for m in range(4):
                    nc.scalar.activation(
                        qks_t[m][:, 512 * n:512 * (n + 1)],
                        accs[m // 2][:, 512 * (m % 2):512 * (m % 2 + 1)],
                        mybir.ActivationFunctionType.Identity,
                        bias=bqk_sb[m][:])
            for m in range(4):
                qks = qks_t[m]
                # RoPE: qk' = qks*cos2 + shift(qks)*sin2s
                # (pair-swap of adjacent partitions via DVE stream shuffle)
                shf = sb.tile([128, N], BF16, name=f"shf{m}", tag="shf", bufs=2)
                nc.vector.stream_shuffle(shf[:], qks[:], swap_mask)
                t2 = sb.tile([128, N], BF16, name=f"ropetmp{m}", tag="ropetmp", bufs=2)
                nc.vector.tensor_mul(t2[:], shf[:], sin_sb[:])
                if m < 2:
                    qkr = sb.tile([128, N], BF16, name=f"qkr{m}", tag=f"qkr{m}")
                    nc.vector.tensor_mul(qkr[:], qks[:], cos_sb[:])
                    nc.vector.tensor_add(qkr[:], qkr[:], t2[:])
                    q_r.append(qkr)
                else:
                    t1 = sb.tile([128, N], BF16, name=f"ropetc{m}", tag="ropetc",
                                 bufs=2)
                    nc.vector.tensor_mul(t1[:], qks[:], cos_sb[:])
                    h0, h1 = 2 * (m - 2), 2 * (m - 2) + 1
                    nc.vector.tensor_add(k_t[h0][0:64, :], t1[0:64, :],
                                         t2[0:64, :])
                    nc.vector.tensor_add(k_t[h1][64:128, :], t1[64:128, :],
                                         t2[64:128, :])

            # ---- v projection (natural [j, ch] layout, ones col appended per head) ----
            # j-chunks processed in pairs with the matmul stream alternating
            # between the two accumulators: back-to-back matmuls into the
            # same PSUM address serialize (~+330ns each), alternating ones
            # pipeline
            vaug = [None] * NJ
            for jp in range(NJ // 2):
                jcs = (2 * jp, 2 * jp + 1)
                pvs = [ps.tile([128, CL], F32, name=f"pv{jc}", tag="sc",
                               bufs=2) for jc in jcs]
                for kc in range(KC):
                    for a, jc in enumerate(jcs):
                        nc.tensor.matmul(
                            pvs[a][:],
                            xb[kc][:, 128 * jc:128 * (jc + 1)],
                            wv_sb[kc][:],
                            start=(kc == 0), stop=(kc == KC - 1))
                for a, jc in enumerate(jcs):
                    va = sb.tile([128, HL * (D + 1)], BF16, name=f"vaug{jc}",
                                 tag=f"vaug{jc}")
                    nc.vector.memset(va[:, D::D + 1], 1.0)
                    nc.scalar.activation(
                        va.rearrange("p (h e) -> p h e", e=D + 1)[:, :, 0:D],
                        pvs[a].rearrange("p (h e) -> p h e", e=D)[:, :, :],
                        mybir.ActivationFunctionType.Copy)
                    vaug[jc] = va

            # per-partition bias AP used to shift scores before fp16 exp
            eshift = sb.tile([128, 1], F32, name="eshift", tag="eshift")
            nc.vector.memset(eshift[:], -16.0)
            # K=1 ones row used to broadcast denominators across partitions
            ones64 = sb.tile([1, 64], BF16, name="ones64", tag="ones64")
            nc.vector.memset(ones64[:], 1.0)

            # ---- attention + projection + RS, per i-chunk ----
            # last chunk is half-width so the final (exposed) ReduceScatter
            # is half the data
            chunks = [(0, 1024), (1024, 1024)]

            def finalize_head(ih, hl, oacc, o_pair, cw):
                # normalize: o[:, i] / den[i].  Broadcast den across
                # partitions with a K=1 matmul (no DMA: DMA triggers can
                # block an engine queue while collective SDMA is in flight),
                # then reciprocal+mul on 64 partitions.
                den = sb.tile([1, cw], BF16, name=f"den{ih}_{hl}",
                              tag="den", bufs=2)
                nc.scalar.activation(den[:], oacc[64:65, :],
                                     mybir.ActivationFunctionType.Copy)
                rb = ps.tile([64, cw], F32, name=f"rb{ih}_{hl}",
                             tag="oacc", bufs=2)
                for q in range(cw // 512):
                    nc.tensor.matmul(rb[:, 512 * q:512 * (q + 1)],
                                     ones64[:],
                                     den[:, 512 * q:512 * (q + 1)],
                                     start=True, stop=True)
                rr = sb.tile([64, cw], F32, name=f"rr{ih}_{hl}", tag="rr",
                             bufs=2)
                nc.vector.reciprocal_approx_fast(rr[:], rb[:])
                nc.vector.tensor_mul(
                    o_pair[hl // 2][64 * (hl % 2):64 * (hl % 2) + 64, :],
                    oacc[0:64, :], rr[:])

            for ih, (i0, cw) in enumerate(chunks):
                ns = cw // 512
                o_pair = [sb.tile([128, cw], BF16, name=f"opair{ih}_{p}",
                                  tag=f"opair{p}", bufs=2) for p in range(2)]
                if ih > 0:
                    _warm_pe(f"c{ih}", 12)
                pending = None
                for hl in range(4):
                    qT = q_r[hl // 2]
                    kT = k_t[hl]
                    oacc = ps.tile([65, cw], F32, name=f"oacc{ih}_{hl}",
                                   tag="oacc", bufs=2)
                    exs = []

                    def emit_o(jc, oacc=oacc, exs=exs, hl=hl):
                        for q in range(ns):
                            nc.tensor.matmul(
                                oacc[:, 512 * q:512 * (q + 1)],
                                vaug[jc][:, (D + 1) * hl:(D + 1) * (hl + 1)],
                                exs[jc][:, 512 * q:512 * (q + 1)],
                                start=(jc == 0), stop=(jc == NJ - 1))

                    for jc in range(NJ):
                        sc = ps.tile([128, cw], F32, name=f"sc{ih}_{hl}_{jc}",
                                     tag="sc", bufs=2)
                        for q in range(ns):
                            nc.tensor.matmul(
                                sc[:, 512 * q:512 * (q + 1)],
                                kT[:, 128 * jc:128 * (jc + 1)],
                                qT[:, i0 + 512 * q:i0 + 512 * (q + 1)],
                                start=True, stop=True)
                        ex = sb.tile([128, cw], BF16, name=f"ex{ih}_{hl}_{jc}",
                                     tag="ex", bufs=3)
                        # bias shifts all scores so fp16 exp can't overflow
                        # (softmax is shift-invariant, cancels in num/den)
                        nc.scalar.activation(ex[:], sc[:],
                                             mybir.ActivationFunctionType.Exp,
                                             scale=float(1.0 / np.sqrt(D)),
                                             bias=eshift[:])
                        exs.append(ex)
                        # software pipeline: o-matmuls lag one j-chunk, and
                        # the previous head's normalization chain is deferred
                        # into this head's pipeline so the exp stream never
                        # pauses at head boundaries
                        if jc == 1 and pending is not None:
                            finalize_head(*pending)
                            pending = None
                        if jc >= 1:
                            emit_o(jc - 1)
                    emit_o(NJ - 1)
                    pending = (ih, hl, oacc, o_pair, cw)
                # keep the PE clock-gate warm through the last head's
                # normalization chain so the projection starts at full rate
                _warm_pe(f"p{ih}", 12)
                finalize_head(*pending)
                pending = None

                # out-projection partial for this i-chunk; mc-outer so the
                # first half of the output channels completes first and its
                # ReduceScatter can fire while the second half projects
                rs_in = dram.tile([C, cw], BF16, name=f"rsin{ih}", tag=f"rsin{ih}")
                half_outs = []
                for half in range(2):
                    for mc in range(4 * half, 4 * half + 4):
                        for n2 in range(ns):
                            isl = slice(512 * n2, 512 * (n2 + 1))
                            pp = ps.tile([128, 512], F32,
                                         name=f"pp{ih}_{n2}_{mc}",
                                         tag="sc" if mc % 2 == 0 else "oacc",
                                         bufs=2)
                            for p in range(2):
                                nc.tensor.matmul(
                                    pp[:],
                                    wproj_sb[p][:, 128 * mc:128 * (mc + 1)],
                                    o_pair[p][:, isl],
                                    start=(p == 0), stop=(p == 1))
                            po = sb.tile([128, 512], BF16,
                                         name=f"po{ih}_{n2}_{mc}",
                                         tag="po", bufs=4)
                            # alternate evict engine so slots recycle 2x faster
                            if mc % 2 == 0:
                                nc.vector.tensor_copy(po[:], pp[:])
                            else:
                                nc.scalar.activation(
                                    po[:], pp[:],
                                    mybir.ActivationFunctionType.Copy)
                            nc.sync.dma_start(
                                rs_in[128 * mc:128 * (mc + 1), isl], po[:])
                    rs_out_h = dram.tile([512 // group_size, cw], BF16,
                                         name=f"rsout{ih}_{half}",
                                         tag=f"rsout{ih}_{half}")
                    nc.gpsimd.collective_compute(
                        "ReduceScatter", mybir.AluOpType.add,
                        replica_groups=groups,
                        ins=[rs_in[512 * half:512 * (half + 1), :]],
                        outs=[rs_out_h[:]])
                    half_outs.append(rs_out_h)

                # RS result + bias -> output, emitted per chunk so a later
                # chunk's collective trigger can't sit ahead of this chunk's
                # readback in the gpsimd queue.  gpsimd DMA: RS-dependent
                # reads must stay out of the sync queue mid-kernel (head-of-
                # line blocking); the last chunk uses the then-idle sync.
                is_last = ih == len(chunks) - 1
                deng = nc.sync if is_last else nc.gpsimd
                half_rows = 512 // group_size
                for half in range(2):
                    for sub in range(half_rows // 128):
                        m = half * (half_rows // 128) + sub
                        rbk = sb.tile([128, cw], BF16, name=f"rbk{ih}_{m}",
                                      tag="rbk", bufs=2)
                        deng.dma_start(
                            rbk[:],
                            half_outs[half][128 * sub:128 * (sub + 1), :])
                        fo = sb.tile([128, cw], F32, name=f"fo{ih}_{m}",
                                     tag="fo", bufs=2)
                        # mid-kernel: gpsimd (slow but hidden under the next
                        # chunk's attention, and RS-ordered so it can't block
                        # attention DVE work). last chunk: DVE (fast, exposed).
                        if is_last:
                            nc.vector.tensor_scalar_add(fo[:], rbk[:],
                                                        beff_sb[m][:])
                        else:
                            nc.gpsimd.tensor_scalar_add(fo[:], rbk[:],
                                                        beff_sb[m][:])
                        deng.dma_start(
                            out.ap()[128 * m:128 * (m + 1), i0:i0 + cw], fo[:])

    nc.compile()
    return nc


def shard_inputs(x, rope, w_qkv, b_qkv, w_proj, b_proj,
                 n_cores=N_CORES, group_size=4):
    """Per-core input maps. Host-side transposes/casts are part of sharding."""
    rs_out_rows = C // group_size
    # fold the v-bias through the projection into an effective output bias
    b_v = b_qkv[2 * C:3 * C]
    b_eff = (b_proj + b_v @ w_proj.T).astype(np.float32)   # [C]

    in_maps = []
    for c in range(n_cores):
        b = (c // group_size) % B
        g = c % group_size
        heads = range(HL * g, HL * g + HL)

        xTb = np.ascontiguousarray(x[b].T).astype(BF)            # [C, N]

        cosT = rope[b].T[:D, :]                                   # [64, N]
        sinT = rope[b].T[D:, :]
        cos2 = np.vstack([cosT, cosT]).astype(BF)                 # [128, N]
        sgn = np.where(np.arange(128) % 2 == 0, -1.0, 1.0)[:, None]
        sin2s = (np.vstack([sinT, sinT]) * sgn).astype(BF)        # [128, N]

        # qk weight rows ordered [q_h0..q_h3, k_h0..k_h3]
        qk_rows = []
        bqk_rows = []
        for h in heads:
            qk_rows.append(w_qkv[D * h:D * (h + 1), :])           # q rows
            bqk_rows.append(b_qkv[D * h:D * (h + 1)])
        for h in heads:
            qk_rows.append(w_qkv[C + D * h:C + D * (h + 1), :])   # k rows
            bqk_rows.append(b_qkv[C + D * h:C + D * (h + 1)])
        wqk = np.vstack(qk_rows)                                  # [512, C]
        wqkT = np.ascontiguousarray(wqk.T).astype(BF)             # [C, 512]
        bqk_v = np.concatenate(bqk_rows).astype(np.float32)[:, None]

        h0 = HL * g
        wv = w_qkv[2 * C + D * h0:2 * C + D * h0 + CL, :]          # [256, C]
        wvT = np.ascontiguousarray(wv.T).astype(BF)                # [C, 256]

        wp = w_proj[:, D * h0:D * h0 + CL]                         # [C, 256]
        wprojT = np.ascontiguousarray(wp.T).astype(BF)             # [256, C]

        # each chunk's RS is split into two half-channel collectives, so
        # rank r receives channels [128r:128r+128) and [512+128r:512+128r+128)
        # (for group_size=4; generally rows r*512/gs of each half)
        r = c % group_size
        hr = 512 // group_size
        beff_shard = np.concatenate(
            [b_eff[hr * r:hr * (r + 1)],
             b_eff[512 + hr * r:512 + hr * (r + 1)]]).astype(
                 np.float32)[:, None]

        in_maps.append({
            "xT": xTb, "cos2": cos2, "sin2s": sin2s,
            "wqkT": wqkT, "bqk": bqk_v, "wvT": wvT,
            "wprojT": wprojT, "beff": beff_shard,
        })
    return in_maps


def assemble(results, n_cores=N_CORES, group_size=4):
    rs_out_rows = C // group_size
    hr = 512 // group_size
    out = np.empty((B, N, C), dtype=np.float32)
    for c in range(n_cores):
        b = (c // group_size) % B
        r = c % group_size
        outT_shard = results[c]["out"]                 # [rs_out_rows, N] f32
        out[b, :, hr * r:hr * (r + 1)] = outT_shard[:hr].T
        out[b, :, 512 + hr * r:512 + hr * (r + 1)] = outT_shard[hr:].T
    return out


_NC_CACHE = {}


def _get_nc():
    if "nc" not in _NC_CACHE:
        _NC_CACHE["nc"] = build_kernel()
    return _NC_CACHE["nc"]


def _run(inputs, trace=False, tmpdir=None):
    nc = _get_nc()
    inputs = {k: np.asarray(v) for k, v in inputs.items()}
    in_maps = shard_inputs(**inputs)
    res = run_bass_kernel_spmd(nc, in_maps, core_ids=list(range(N_CORES)),
                               trace=trace, tmpdir=tmpdir)
    return assemble(res.results), res


def kernel(**inputs):
    out, _ = _run(inputs)
    return out



# revision 3
# speedup vs baseline: 1.0224x; 1.0224x over previous
"""Multi-head attention (B=2, N=2048, C=1024, H=16, D=64) on 8 TRN2 NeuronCores.

Sharding: core c = (batch b = c//4) x (head-group g = c%4 -> heads 4g..4g+3).
Data parallel on B, tensor parallel on heads; fp16 ReduceScatter of the
out-projection partials within each 4-core batch group.

v2 vs baseline:
- scores matmuls row-tiled: heads are processed in pairs (h0,h1)/(h2,h3)
  with q/k packed [h_even rows 0:64, h_odd rows 64:128]; the two K=64
  score matmuls of a pair run CONCURRENTLY on distinct PE row-groups
  (tile_position (0,0)/(64,0)) -> 2x scores throughput vs zero-padded
  K=128 (validated: 141ns vs 306ns per head-chunk).
- softmax exp split across two engines: ACT computes real exp for the
  even head; DVE computes a Schraudolph bitcast exp (i16 = s*A+B viewed
  as fp16) for the odd head at the same rate (~690ns/[128,512] tile,
  max 3.9% elem err, washes out in softmax normalization). Each softmax
  column is computed wholly by one engine so shifts don't need to match.
- i-chunks of 512 columns (4 chunks) so the output ReduceScatter
  pipeline starts earlier and the exposed tail is 1/4 the size.
- output bias folded into rank-0 cores' pre-RS partials (zero-padded
  beff input on other ranks) so the post-RS readback is a plain
  cast+store.

Everything on device stays transposed ([channel, position]); the host
pre-transposes inputs and post-transposes the output.
"""

import numpy as np

import concourse.bacc as bacc
import concourse.tile as tile
import concourse.mybir as mybir
from concourse.bass_utils import run_bass_kernel_spmd

B, N, C, H = 2, 2048, 1024, 16
D = C // H          # 64
HL = H // 4         # 4 heads per core
CL = HL * D         # 256 local channels
N_CORES = 8
GROUPS = [[0, 1, 2, 3], [4, 5, 6, 7]]

F32 = mybir.dt.float32
F16 = mybir.dt.float16
BF16 = mybir.dt.bfloat16
I16 = mybir.dt.int16
BF = np.float16

KC = C // 128       # 8  K-chunks of the input channel dim
NJ = N // 128       # 16 128-row j-chunks
CW = 512            # i-chunk width
NCH = N // CW       # 4 i-chunks

LOG2E = 1.4426950408889634
SCALE = 1.0 / np.sqrt(D)                      # 0.125
# Schraudolph exp into BF16 bits: i16 = round(x*scale*log2e*128 + 127*128-c).
# bf16 (8-bit exponent) covers e^x for scaled scores in [-28, +26] (randn
# rope makes q/k product-normal, so scores reach ~14 sigma) with no shift;
# fp16 cannot. c=7.3 centers the +-3% linearization error, +0.5 centers
# the truncating f32->i16 convert.
A_SCH = SCALE * LOG2E * 128.0
B_SCH = 127.0 * 128.0 - 7.3 + 0.5
# ACT-side exp uses the same zero shift so ACT_BOTH tiles interchange with
# DVE tiles inside one softmax column group; outputs are bf16 so e^26 fits.
ACT_BIAS = 0.0
ACT_BOTH = (8,)                               # j-chunks where ACT also takes the DVE head


def build_kernel(n_cores=N_CORES, groups=GROUPS):
    group_size = len(groups[0])
    rs_out_rows = C // group_size             # 256

    nc = bacc.Bacc("TRN2", target_bir_lowering=False, debug=False,
                   num_devices=n_cores)

    xT = nc.declare_dram_parameter("xT", [C, N], F16, isOutput=False)
    cos2 = nc.declare_dram_parameter("cos2", [128, N], F16, isOutput=False)
    sin2s = nc.declare_dram_parameter("sin2s", [128, N], F16, isOutput=False)
    wqkT = nc.declare_dram_parameter("wqkT", [C, 2 * CL], F16, isOutput=False)
    bqk = nc.declare_dram_parameter("bqk", [2 * CL, 1], F32, isOutput=False)
    wvT = nc.declare_dram_parameter("wvT", [C, CL], F16, isOutput=False)
    wprojT = nc.declare_dram_parameter("wprojT", [CL, C], F16, isOutput=False)
    beff = nc.declare_dram_parameter("beff", [C, 1], F32, isOutput=False)
    out = nc.declare_dram_parameter("out", [rs_out_rows, N], F32, isOutput=True)

    with tile.TileContext(nc) as tc:
        with tc.tile_pool(name="dram", bufs=1, space="DRAM") as dram, \
             tc.tile_pool(name="sbuf", bufs=1) as sb, \
             tc.tile_pool(name="psum", bufs=1, space="PSUM") as ps:

            PTAGS = ["pA", "pB", "oA", "oB"]

            # tile for clock-warming matmuls
            warm = sb.tile([128, 128], F16, name="warm", tag="warm")
            nc.vector.memset(warm[:], 0.001)

            def _warm_pe(tag, n):
                wps = [ps.tile([128, 64], F32, name=f"warmp{tag}_{a}",
                               tag=PTAGS[a], bufs=2) for a in range(2)]
                for r in range(n):
                    nc.tensor.matmul(wps[r % 2][:], warm[:], warm[:, :64],
                                     start=True, stop=True)

            _warm_pe("s", 24)

            # ---- load inputs ----
            xb, wqk_sb = [], []
            for kc in range(KC):
                t = sb.tile([128, 2 * CL], F16, name=f"wqk{kc}", tag=f"wqk{kc}")
                eng = nc.scalar if kc % 2 == 0 else nc.sync
                eng.dma_start(t[:], wqkT.ap()[128 * kc:128 * (kc + 1), :])
                wqk_sb.append(t)
                t = sb.tile([128, N], F16, name=f"xb{kc}", tag=f"xb{kc}")
                eng = nc.sync if kc % 2 == 0 else nc.scalar
                eng.dma_start(t[:], xT.ap()[128 * kc:128 * (kc + 1), :])
                xb.append(t)
            wv_sb = []
            for kc in range(KC):
                t = sb.tile([128, CL], F16, name=f"wv{kc}", tag=f"wv{kc}")
                nc.sync.dma_start(t[:], wvT.ap()[128 * kc:128 * (kc + 1), :])
                wv_sb.append(t)
            cos_sb = sb.tile([128, N], F16, name="cos_sb", tag="cos_sb")
            nc.sync.dma_start(cos_sb[:], cos2.ap())
            sin_sb = sb.tile([128, N], F16, name="sin_sb", tag="sin_sb")
            nc.scalar.dma_start(sin_sb[:], sin2s.ap())
            bqk_sb = []
            for m in range(4):
                t = sb.tile([128, 1], F32, name=f"bqk{m}", tag=f"bqk{m}")
                nc.sync.dma_start(t[:], bqk.ap()[128 * m:128 * (m + 1), :])
                bqk_sb.append(t)
            wproj_sb = []
            for p in range(2):
                t = sb.tile([128, C], F16, name=f"wproj{p}", tag=f"wproj{p}")
                nc.sync.dma_start(t[:], wprojT.ap()[128 * p:128 * (p + 1), :])
                wproj_sb.append(t)
            beff_sb = []
            for mc in range(8):
                t = sb.tile([128, 1], F32, name=f"beff{mc}", tag=f"beff{mc}")
                nc.sync.dma_start(t[:], beff.ap()[128 * mc:128 * (mc + 1), :])
                beff_sb.append(t)

            # constants; ACT exp-table preload happens on the first dummy exp
            eshift = sb.tile([128, 1], F32, name="eshift", tag="eshift")
            nc.vector.memset(eshift[:], ACT_BIAS)
            ones64 = sb.tile([1, 64], BF16, name="ones64", tag="ones64")
            nc.vector.memset(ones64[:], 1.0)
            dummy = sb.tile([128, 1], F16, name="dummy", tag="dummy")
            nc.scalar.activation(dummy[:], eshift[:],
                                 mybir.ActivationFunctionType.Exp,
                                 scale=1.0, bias=eshift[:])

            # vaug ones columns set once, up front, on the idle gpsimd engine
            vaug = []
            for jc in range(NJ):
                va = sb.tile([128, HL * (D + 1)], BF16, name=f"vaug{jc}",
                             tag=f"vaug{jc}")
                nc.gpsimd.memset(va[:, D::D + 1], 1.0)
                vaug.append(va)

            # ---- qk projection ----
            # m chunk rows: m=0:[q_h0,q_h1] m=1:[q_h2,q_h3] m=2:[k_h0,k_h1]
            # m=3:[k_h2,k_h3]; pair p uses q rows of m=p, k rows of m=2+p.
            qks_t = [sb.tile([128, N], F16, name=f"qks{m}", tag=f"qks{m}")
                     for m in range(4)]
            for n in range(NCH):
                accs = [ps.tile([128, CW], F32, name=f"qacc{n}_{m}",
                                tag=PTAGS[m], bufs=2) for m in range(4)]
                for kc in range(KC):
                    for m in range(4):
                        nc.tensor.matmul(
                            accs[m][:],
                            wqk_sb[kc][:, 128 * m:128 * (m + 1)],
                            xb[kc][:, CW * n:CW * (n + 1)],
                            start=(kc == 0), stop=(kc == KC - 1))
                for m in range(4):
                    dst = qks_t[m][:, CW * n:CW * (n + 1)]
                    if m % 2 == 0:
                        nc.scalar.activation(
                            dst, accs[m][:],
                            mybir.ActivationFunctionType.Identity,
                            bias=bqk_sb[m][:])
                    else:
                        nc.vector.tensor_scalar_add(dst, accs[m][:],
                                                    bqk_sb[m][:])

            # ---- RoPE ----  qk' = qks*cos2 + pairswap(qks)*sin2s
            # order pair-0's q (m=0) and k (m=2) first so attention can start
            q_r = [None, None]   # packed [q_h2p; q_h2p+1]
            k_p = [None, None]   # packed [k_h2p; k_h2p+1]
            swap_mask = [i ^ 1 for i in range(32)]
            for m in (0, 2, 1, 3):
                qks = qks_t[m]
                shf = sb.tile([128, N], F16, name=f"shf{m}", tag="shf", bufs=2)
                nc.vector.stream_shuffle(shf[:], qks[:], swap_mask)
                t2 = sb.tile([128, N], F16, name=f"ropetmp{m}", tag="ropetmp",
                             bufs=2)
                nc.vector.tensor_mul(t2[:], shf[:], sin_sb[:])
                dst = sb.tile([128, N], F16, name=f"qkr{m}", tag=f"qkr{m}")
                nc.vector.tensor_mul(dst[:], qks[:], cos_sb[:])
                nc.vector.tensor_add(dst[:], dst[:], t2[:])
                if m < 2:
                    q_r[m] = dst
                else:
                    k_p[m - 2] = dst

            # ---- v projection ----
            for jp in range(NJ // 2):
                jcs = (2 * jp, 2 * jp + 1)
                pvs = [ps.tile([128, CL], F32, name=f"pv{jc}",
                               tag=PTAGS[a], bufs=2) for a, jc in enumerate(jcs)]
                for kc in range(KC):
                    for a, jc in enumerate(jcs):
                        nc.tensor.matmul(
                            pvs[a][:],
                            xb[kc][:, 128 * jc:128 * (jc + 1)],
                            wv_sb[kc][:],
                            start=(kc == 0), stop=(kc == KC - 1))
                for a, jc in enumerate(jcs):
                    dst = vaug[jc].rearrange("p (h e) -> p h e",
                                             e=D + 1)[:, :, 0:D]
                    src = pvs[a].rearrange("p (h e) -> p h e", e=D)[:, :, :]
                    if a == 0:
                        nc.scalar.activation(
                            dst, src, mybir.ActivationFunctionType.Copy)
                    else:
                        nc.vector.tensor_copy(dst, src)

            # ---- attention, head pairs, i-chunks of 512 ----
            o_pair = {}

            def finalize_head(ih, p, hl, oacc, cw):
                # o[:, q] / den[q]: den -> SBUF (ACT), broadcast across 64
                # partitions via K=1 matmul, fast reciprocal + mul (DVE)
                den = sb.tile([1, cw], BF16, name=f"den{ih}_{hl}",
                              tag="den", bufs=2)
                nc.scalar.activation(den[:], oacc[64:65, :],
                                     mybir.ActivationFunctionType.Copy)
                rb = ps.tile([64, cw], F32, name=f"rb{ih}_{hl}",
                             tag="pA", bufs=2)
                nc.tensor.matmul(rb[:], ones64[:], den[:],
                                 start=True, stop=True)
                rr = sb.tile([64, cw], F32, name=f"rr{ih}_{hl}", tag="rr",
                             bufs=2)
                nc.vector.reciprocal_approx_fast(rr[:], rb[:])
                nc.vector.tensor_mul(
                    o_pair[p][64 * (hl % 2):64 * (hl % 2) + 64, :],
                    oacc[0:64, :], rr[:])

            for ih in range(NCH):
                i0 = ih * CW
                o_pair[0] = sb.tile([128, CW], F16, name=f"op0_{ih}",
                                    tag="opair0", bufs=2)
                o_pair[1] = sb.tile([128, CW], F16, name=f"op1_{ih}",
                                    tag="opair1", bufs=2)
                pending = []
                for p in range(2):
                    hA, hB = 2 * p, 2 * p + 1
                    oaccA = ps.tile([65, CW], F32, name=f"oacc{ih}_{hA}",
                                    tag="oA", bufs=2)
                    oaccB = ps.tile([65, CW], F32, name=f"oacc{ih}_{hB}",
                                    tag="oB", bufs=2)
                    exsA, exsB = [], []

                    def emit_o(jc, oaccA=oaccA, oaccB=oaccB,
                               exsA=exsA, exsB=exsB, hA=hA, hB=hB):
                        nc.tensor.matmul(
                            oaccA[:],
                            vaug[jc][:, (D + 1) * hA:(D + 1) * (hA + 1)],
                            exsA[jc][:],
                            start=(jc == 0), stop=(jc == NJ - 1))
                        nc.tensor.matmul(
                            oaccB[:],
                            vaug[jc][:, (D + 1) * hB:(D + 1) * (hB + 1)],
                            exsB[jc][:],
                            start=(jc == 0), stop=(jc == NJ - 1))

                    for jc in range(NJ):
                        scA = ps.tile([128, CW], F32, name=f"sc{ih}_{hA}_{jc}",
                                      tag="pA", bufs=2)
                        scB = ps.tile([128, CW], F32, name=f"sc{ih}_{hB}_{jc}",
                                      tag="pB", bufs=2)
                        # concurrent row-tiled K=64 pair
                        nc.tensor.matmul(
                            scA[:],
                            k_p[p][0:64, 128 * jc:128 * (jc + 1)],
                            q_r[p][0:64, i0:i0 + CW],
                            start=True, stop=True)
                        nc.tensor.matmul(
                            scB[:],
                            k_p[p][64:128, 128 * jc:128 * (jc + 1)],
                            q_r[p][64:128, i0:i0 + CW],
                            start=True, stop=True)
                        exA = sb.tile([128, CW], BF16, name=f"ex{ih}_{hA}_{jc}",
                                      tag="exA", bufs=3)
                        nc.scalar.activation(exA[:], scA[:],
                                             mybir.ActivationFunctionType.Exp,
                                             scale=float(SCALE),
                                             bias=eshift[:])
                        exB = sb.tile([128, CW], BF16, name=f"ex{ih}_{hB}_{jc}",
                                      tag="exB", bufs=3)
                        if jc in ACT_BOTH:
                            nc.scalar.activation(
                                exB[:], scB[:],
                                mybir.ActivationFunctionType.Exp,
                                scale=float(SCALE), bias=eshift[:])
                        else:
                            nc.vector.tensor_scalar(
                                exB[:].bitcast(I16), scB[:],
                                float(A_SCH), float(B_SCH),
                                mybir.AluOpType.mult, mybir.AluOpType.add)
                        exsA.append(exA)
                        exsB.append(exB)
                        # previous pair's deferred normalization rides inside
                        # this pair's pipeline so the exp stream never pauses
                        if jc == 1 and pending:
                            finalize_head(*pending.pop(0))
                        if jc == 2 and pending:
                            finalize_head(*pending.pop(0))
                        if jc >= 1:
                            emit_o(jc - 1)
                    emit_o(NJ - 1)
                    if p == 0:
                        pending = [(ih, 0, hA, oaccA, CW),
                                   (ih, 0, hB, oaccB, CW)]
                    else:
                        finalize_head(ih, 1, hA, oaccA, CW)
                        finalize_head(ih, 1, hB, oaccB, CW)

                # ---- out-projection partial + RS for this i-chunk ----
                # p=0 accumulation first: o_pair[0] finalized a pair ago, so
                # these 8 matmuls run while DVE finishes pair-1's norm
                rs_in = dram.tile([C, CW], F16, name=f"rsin{ih}",
                                  tag=f"rsin{ih}")
                pps = []
                for mc in range(8):
                    pp = ps.tile([128, CW], F32, name=f"pp{ih}_{mc}",
                                 tag=PTAGS[mc % 4], bufs=2)
                    nc.tensor.matmul(pp[:],
                                     wproj_sb[0][:, 128 * mc:128 * (mc + 1)],
                                     o_pair[0][:],
                                     start=True, stop=False)
                    pps.append(pp)
                half_outs = []
                for half in range(2):
                    for mc in range(4 * half, 4 * half + 4):
                        nc.tensor.matmul(pps[mc][:],
                                         wproj_sb[1][:, 128 * mc:128 * (mc + 1)],
                                         o_pair[1][:],
                                         start=False, stop=True)
                        po = sb.tile([128, CW], F16, name=f"po{ih}_{mc}",
                                     tag="po", bufs=4)
                        # bias pre-folded into rank-0 cores' partials
                        # (beff input is zeros on other ranks)
                        if mc % 2 == 0:
                            nc.vector.tensor_scalar_add(po[:], pps[mc][:],
                                                        beff_sb[mc][:])
                        else:
                            nc.scalar.activation(
                                po[:], pps[mc][:],
                                mybir.ActivationFunctionType.Identity,
                                bias=beff_sb[mc][:])
                        nc.sync.dma_start(
                            rs_in[128 * mc:128 * (mc + 1), :], po[:])
                    rs_out_h = dram.tile([512 // group_size, CW], F16,
                                         name=f"rsout{ih}_{half}",
                                         tag=f"rsout{ih}_{half}")
                    nc.gpsimd.collective_compute(
                        "ReduceScatter", mybir.AluOpType.add,
                        replica_groups=groups,
                        ins=[rs_in[512 * half:512 * (half + 1), :]],
                        outs=[rs_out_h[:]])
                    half_outs.append(rs_out_h)

                # RS result -> cast f32 -> output. gpsimd mid-kernel (slow but
                # hidden and RS-ordered); sync+DVE for the exposed last chunk.
                is_last = ih == NCH - 1
                deng = nc.sync if is_last else nc.gpsimd
                for half in range(2):
                    rbk = sb.tile([128, CW], F16, name=f"rbk{ih}_{half}",
                                  tag="rbk", bufs=2)
                    deng.dma_start(rbk[:], half_outs[half][:])
                    fo = sb.tile([128, CW], F32, name=f"fo{ih}_{half}",
                                 tag="fo", bufs=2)
                    if is_last:
                        nc.vector.tensor_copy(fo[:], rbk[:])
                    else:
                        nc.gpsimd.tensor_scalar_add(fo[:], rbk[:], 0.0)
                    deng.dma_start(
                        out.ap()[128 * half:128 * (half + 1), i0:i0 + CW],
                        fo[:])

    nc.compile()
    return nc


def shard_inputs(x, rope, w_qkv, b_qkv, w_proj, b_proj,
                 n_cores=N_CORES, group_size=4):
    """Per-core input maps. Host-side transposes/casts are part of sharding."""
    # fold the v-bias through the projection into an effective output bias
    b_v = b_qkv[2 * C:3 * C]
    b_eff = (b_proj + b_v @ w_proj.T).astype(np.float32)   # [C]

    in_maps = []
    for c in range(n_cores):
        b = (c // group_size) % B
        g = c % group_size
        heads = range(HL * g, HL * g + HL)

        xTb = np.ascontiguousarray(x[b].T).astype(BF)            # [C, N]

        cosT = rope[b].T[:D, :]                                   # [64, N]
        sinT = rope[b].T[D:, :]
        cos2 = np.vstack([cosT, cosT]).astype(BF)                 # [128, N]
        sgn = np.where(np.arange(128) % 2 == 0, -1.0, 1.0)[:, None]
        sin2s = (np.vstack([sinT, sinT]) * sgn).astype(BF)        # [128, N]

        # qk weight rows ordered [q_h0..q_h3, k_h0..k_h3]
        qk_rows = []
        bqk_rows = []
        for h in heads:
            qk_rows.append(w_qkv[D * h:D * (h + 1), :])           # q rows
            bqk_rows.append(b_qkv[D * h:D * (h + 1)])
        for h in heads:
            qk_rows.append(w_qkv[C + D * h:C + D * (h + 1), :])   # k rows
            bqk_rows.append(b_qkv[C + D * h:C + D * (h + 1)])
        wqk = np.vstack(qk_rows)                                  # [512, C]
        wqkT = np.ascontiguousarray(wqk.T).astype(BF)             # [C, 512]
        bqk_v = np.concatenate(bqk_rows).astype(np.float32)[:, None]

        h0 = HL * g
        wv = w_qkv[2 * C + D * h0:2 * C + D * h0 + CL, :]          # [256, C]
        wvT = np.ascontiguousarray(wv.T).astype(BF)                # [C, 256]

        wp = w_proj[:, D * h0:D * h0 + CL]                         # [C, 256]
        wprojT = np.ascontiguousarray(wp.T).astype(BF)             # [256, C]

        # bias enters via rank 0's pre-RS partials; zeros elsewhere
        if c % group_size == 0:
            beff_full = b_eff.reshape(C, 1).astype(np.float32)
        else:
            beff_full = np.zeros((C, 1), np.float32)

        in_maps.append({
            "xT": xTb, "cos2": cos2, "sin2s": sin2s,
            "wqkT": wqkT, "bqk": bqk_v, "wvT": wvT,
            "wprojT": wprojT, "beff": beff_full,
        })
    return in_maps


def assemble(results, n_cores=N_CORES, group_size=4):
    hr = 512 // group_size
    out = np.empty((B, N, C), dtype=np.float32)
    for c in range(n_cores):
        b = (c // group_size) % B
        r = c % group_size
        outT_shard = results[c]["out"]                 # [256, N] f32
        out[b, :, hr * r:hr * (r + 1)] = outT_shard[:hr].T
        out[b, :, 512 + hr * r:512 + hr * (r + 1)] = outT_shard[hr:].T
    return out


_NC_CACHE = {}


def _get_nc():
    if "nc" not in _NC_CACHE:
        _NC_CACHE["nc"] = build_kernel()
    return _NC_CACHE["nc"]


def _run(inputs, trace=False, tmpdir=None):
    nc = _get_nc()
    inputs = {k: np.asarray(v) for k, v in inputs.items()}
    in_maps = shard_inputs(**inputs)
    res = run_bass_kernel_spmd(nc, in_maps, core_ids=list(range(N_CORES)),
                               trace=trace, tmpdir=tmpdir)
    return assemble(res.results), res


def kernel(**inputs):
    out, _ = _run(inputs)
    return out


# revision 7
# speedup vs baseline: 1.1072x; 1.0829x over previous
"""Multi-head attention (B=2, N=2048, C=1024, H=16, D=64) on 8 TRN2 NeuronCores.

Sharding: core c = (batch b = c//4) x (head-group g = c%4 -> heads 4g..4g+3).
Data parallel on B, tensor parallel on heads; fp16 ReduceScatter of the
out-projection partials within each 4-core batch group.

v2 vs baseline:
- scores matmuls row-tiled: heads are processed in pairs (h0,h1)/(h2,h3)
  with q/k packed [h_even rows 0:64, h_odd rows 64:128]; the two K=64
  score matmuls of a pair run CONCURRENTLY on distinct PE row-groups
  (tile_position (0,0)/(64,0)) -> 2x scores throughput vs zero-padded
  K=128 (validated: 141ns vs 306ns per head-chunk).
- softmax exp split across two engines: ACT computes real exp for the
  even head; DVE computes a Schraudolph bitcast exp (i16 = s*A+B viewed
  as fp16) for the odd head at the same rate (~690ns/[128,512] tile,
  max 3.9% elem err, washes out in softmax normalization). Each softmax
  column is computed wholly by one engine so shifts don't need to match.
- i-chunks of 512 columns (4 chunks) so the output ReduceScatter
  pipeline starts earlier and the exposed tail is 1/4 the size.
- output bias folded into rank-0 cores' pre-RS partials (zero-padded
  beff input on other ranks) so the post-RS readback is a plain
  cast+store.

Everything on device stays transposed ([channel, position]); the host
pre-transposes inputs and post-transposes the output.
"""

import numpy as np

import concourse.bacc as bacc
import concourse.tile as tile
import concourse.mybir as mybir
from concourse.bass_utils import run_bass_kernel_spmd

B, N, C, H = 2, 2048, 1024, 16
D = C // H          # 64
HL = H // 4         # 4 heads per core
CL = HL * D         # 256 local channels
N_CORES = 8
GROUPS = [[0, 1, 2, 3], [4, 5, 6, 7]]

F32 = mybir.dt.float32
F16 = mybir.dt.float16
BF16 = mybir.dt.bfloat16
I16 = mybir.dt.int16
BF = np.float16

KC = C // 128       # 8  K-chunks of the input channel dim
NJ = N // 128       # 16 128-row j-chunks
CW = 512            # i-chunk width
NCH = N // CW       # 4 i-chunks

LOG2E = 1.4426950408889634
SCALE = 1.0 / np.sqrt(D)                      # 0.125
# Schraudolph exp into BF16 bits: i16 = round(x*scale*log2e*128 + 127*128-c).
# bf16 (8-bit exponent) covers e^x for scaled scores in [-28, +26] (randn
# rope makes q/k product-normal, so scores reach ~14 sigma) with no shift;
# fp16 cannot. c=7.3 centers the +-3% linearization error, +0.5 centers
# the truncating f32->i16 convert.
A_SCH = SCALE * LOG2E * 128.0
B_SCH = 127.0 * 128.0 - 7.3 + 0.5
# ACT-side exp uses the same zero shift so ACT_BOTH tiles interchange with
# DVE tiles inside one softmax column group; outputs are bf16 so e^26 fits.
ACT_BIAS = 0.0
ACT_BOTH = (8,)                               # j-chunks where ACT also takes the DVE head


def build_kernel(n_cores=N_CORES, groups=GROUPS):
    group_size = len(groups[0])
    rs_out_rows = C // group_size             # 256

    nc = bacc.Bacc("TRN2", target_bir_lowering=False, debug=False,
                   num_devices=n_cores)

    xT = nc.declare_dram_parameter("xT", [C, N], F16, isOutput=False)
    cos2 = nc.declare_dram_parameter("cos2", [128, N], F16, isOutput=False)
    sin2s = nc.declare_dram_parameter("sin2s", [128, N], F16, isOutput=False)
    wqkT = nc.declare_dram_parameter("wqkT", [C, 2 * CL], F16, isOutput=False)
    bqk = nc.declare_dram_parameter("bqk", [2 * CL, 1], F32, isOutput=False)
    wvT = nc.declare_dram_parameter("wvT", [C, CL], F16, isOutput=False)
    wprojT = nc.declare_dram_parameter("wprojT", [CL, C], F16, isOutput=False)
    beff = nc.declare_dram_parameter("beff", [C, 1], F32, isOutput=False)
    out = nc.declare_dram_parameter("out", [rs_out_rows, N], F16, isOutput=True)

    with tile.TileContext(nc) as tc:
        with tc.tile_pool(name="dram", bufs=1, space="DRAM") as dram, \
             tc.tile_pool(name="sbuf", bufs=1) as sb, \
             tc.tile_pool(name="psum", bufs=1, space="PSUM") as ps:

            PTAGS = ["pA", "pB", "oA", "oB"]

            # tile for clock-warming matmuls
            warm = sb.tile([128, 128], F16, name="warm", tag="warm")
            nc.vector.memset(warm[:], 0.001)

            def _warm_pe(tag, n):
                wps = [ps.tile([128, 64], F32, name=f"warmp{tag}_{a}",
                               tag=PTAGS[a], bufs=2) for a in range(2)]
                for r in range(n):
                    nc.tensor.matmul(wps[r % 2][:], warm[:], warm[:, :64],
                                     start=True, stop=True)

            _warm_pe("s", 24)

            # ---- load inputs ----
            xb, wqk_sb = [], []
            for kc in range(KC):
                t = sb.tile([128, 2 * CL], F16, name=f"wqk{kc}", tag=f"wqk{kc}")
                eng = nc.scalar if kc % 2 == 0 else nc.sync
                eng.dma_start(t[:], wqkT.ap()[128 * kc:128 * (kc + 1), :])
                wqk_sb.append(t)
                t = sb.tile([128, N], F16, name=f"xb{kc}", tag=f"xb{kc}")
                eng = nc.sync if kc % 2 == 0 else nc.scalar
                eng.dma_start(t[:], xT.ap()[128 * kc:128 * (kc + 1), :])
                xb.append(t)
            wv_sb = []
            for kc in range(KC):
                t = sb.tile([128, CL], F16, name=f"wv{kc}", tag=f"wv{kc}")
                nc.sync.dma_start(t[:], wvT.ap()[128 * kc:128 * (kc + 1), :])
                wv_sb.append(t)
            cos_sb = sb.tile([128, N], F16, name="cos_sb", tag="cos_sb")
            nc.sync.dma_start(cos_sb[:], cos2.ap())
            sin_sb = sb.tile([128, N], F16, name="sin_sb", tag="sin_sb")
            nc.scalar.dma_start(sin_sb[:], sin2s.ap())
            bqk_sb = []
            for m in range(4):
                t = sb.tile([128, 1], F32, name=f"bqk{m}", tag=f"bqk{m}")
                nc.sync.dma_start(t[:], bqk.ap()[128 * m:128 * (m + 1), :])
                bqk_sb.append(t)
            wproj_sb = []
            for p in range(2):
                t = sb.tile([128, C], F16, name=f"wproj{p}", tag=f"wproj{p}")
                nc.sync.dma_start(t[:], wprojT.ap()[128 * p:128 * (p + 1), :])
                wproj_sb.append(t)
            beff_sb = []
            for mc in range(8):
                t = sb.tile([128, 1], F32, name=f"beff{mc}", tag=f"beff{mc}")
                nc.sync.dma_start(t[:], beff.ap()[128 * mc:128 * (mc + 1), :])
                beff_sb.append(t)

            # constants; ACT exp-table preload happens on the first dummy exp
            eshift = sb.tile([128, 1], F32, name="eshift", tag="eshift")
            nc.vector.memset(eshift[:], ACT_BIAS)
            ones64 = sb.tile([1, 64], BF16, name="ones64", tag="ones64")
            nc.vector.memset(ones64[:], 1.0)
            dummy = sb.tile([128, 1], F16, name="dummy", tag="dummy")
            nc.scalar.activation(dummy[:], eshift[:],
                                 mybir.ActivationFunctionType.Exp,
                                 scale=1.0, bias=eshift[:])

            # vaug ones columns set once, up front, on the idle gpsimd engine
            vaug = []
            for jc in range(NJ):
                va = sb.tile([128, HL * (D + 1)], BF16, name=f"vaug{jc}",
                             tag=f"vaug{jc}")
                nc.gpsimd.memset(va[:, D::D + 1], 1.0)
                vaug.append(va)

            # ---- qk projection + RoPE, m-outer ----
            # m chunk rows: m=0:[q_h0,q_h1] m=1:[q_h2,q_h3] m=2:[k_h0,k_h1]
            # m=3:[k_h2,k_h3]; pair p uses q rows of m=p, k rows of m=2+p.
            # m-outer with n innermost so 4 consecutive matmuls share the
            # stationary wqk window (amortized LDWEIGHTS); RoPE for each m
            # is emitted right after its evictions so the DVE starts the
            # rotation ~25us earlier than a separate RoPE phase would.
            qks_t = [sb.tile([128, N], F16, name=f"qks{m}", tag=f"qks{m}")
                     for m in range(4)]
            q_r = [None, None]   # packed [q_h2p; q_h2p+1]
            k_p = [None, None]   # packed [k_h2p; k_h2p+1]
            swap_mask = [i ^ 1 for i in range(32)]
            for m in (0, 2, 1, 3):
                accs = [ps.tile([128, CW], F32, name=f"qacc{m}_{n}",
                                tag=PTAGS[n], bufs=2) for n in range(4)]
                for kc in range(KC):
                    for n in range(4):
                        nc.tensor.matmul(
                            accs[n][:],
                            wqk_sb[kc][:, 128 * m:128 * (m + 1)],
                            xb[kc][:, CW * n:CW * (n + 1)],
                            start=(kc == 0), stop=(kc == KC - 1))
                for n in range(4):
                    # evictions on ACT so the DVE is free for RoPE
                    nc.scalar.activation(
                        qks_t[m][:, CW * n:CW * (n + 1)], accs[n][:],
                        mybir.ActivationFunctionType.Identity,
                        bias=bqk_sb[m][:])
                # RoPE: qk' = qks*cos2 + pairswap(qks)*sin2s
                qks = qks_t[m]
                shf = sb.tile([128, N], F16, name=f"shf{m}", tag="shf", bufs=2)
                nc.vector.stream_shuffle(shf[:], qks[:], swap_mask)
                t2 = sb.tile([128, N], F16, name=f"ropetmp{m}", tag="ropetmp",
                             bufs=2)
                nc.vector.tensor_mul(t2[:], shf[:], sin_sb[:])
                dst = sb.tile([128, N], F16, name=f"qkr{m}", tag=f"qkr{m}")
                nc.vector.tensor_mul(dst[:], qks[:], cos_sb[:])
                nc.vector.tensor_add(dst[:], dst[:], t2[:])
                if m < 2:
                    q_r[m] = dst
                else:
                    k_p[m - 2] = dst

            # ---- v projection ----
            for jp in range(NJ // 2):
                jcs = (2 * jp, 2 * jp + 1)
                pvs = [ps.tile([128, CL], F32, name=f"pv{jc}",
                               tag=PTAGS[a], bufs=2) for a, jc in enumerate(jcs)]
                for kc in range(KC):
                    for a, jc in enumerate(jcs):
                        nc.tensor.matmul(
                            pvs[a][:],
                            xb[kc][:, 128 * jc:128 * (jc + 1)],
                            wv_sb[kc][:],
                            start=(kc == 0), stop=(kc == KC - 1))
                for a, jc in enumerate(jcs):
                    # both evictions on ACT: the DVE is busy with RoPE here
                    dst = vaug[jc].rearrange("p (h e) -> p h e",
                                             e=D + 1)[:, :, 0:D]
                    src = pvs[a].rearrange("p (h e) -> p h e", e=D)[:, :, :]
                    nc.scalar.activation(
                        dst, src, mybir.ActivationFunctionType.Copy)

            # ---- attention, head pairs, i-chunks of 512 ----
            o_pair = {}

            def finalize_head(ih, p, hl, oacc, cw):
                # o[:, q] / den[q]: den -> SBUF (ACT), broadcast across 64
                # partitions via K=1 matmul, fast reciprocal + mul (DVE)
                den = sb.tile([1, cw], BF16, name=f"den{ih}_{hl}",
                              tag="den", bufs=2)
                nc.scalar.activation(den[:], oacc[64:65, :],
                                     mybir.ActivationFunctionType.Copy)
                rb = ps.tile([64, cw], F32, name=f"rb{ih}_{hl}",
                             tag="pA", bufs=2)
                nc.tensor.matmul(rb[:], ones64[:], den[:],
                                 start=True, stop=True)
                rr = sb.tile([64, cw], F32, name=f"rr{ih}_{hl}", tag="rr",
                             bufs=2)
                nc.vector.reciprocal_approx_fast(rr[:], rb[:])
                nc.vector.tensor_mul(
                    o_pair[p][64 * (hl % 2):64 * (hl % 2) + 64, :],
                    oacc[0:64, :], rr[:])

            for ih in range(NCH):
                i0 = ih * CW
                o_pair[0] = sb.tile([128, CW], F16, name=f"op0_{ih}",
                                    tag="opair0", bufs=2)
                o_pair[1] = sb.tile([128, CW], F16, name=f"op1_{ih}",
                                    tag="opair1", bufs=2)
                pending = []
                for p in range(2):
                    hA, hB = 2 * p, 2 * p + 1
                    oaccA = ps.tile([65, CW], F32, name=f"oacc{ih}_{hA}",
                                    tag="oA", bufs=2)
                    oaccB = ps.tile([65, CW], F32, name=f"oacc{ih}_{hB}",
                                    tag="oB", bufs=2)
                    exsA, exsB = [], []

                    def emit_o(jc, oaccA=oaccA, oaccB=oaccB,
                               exsA=exsA, exsB=exsB, hA=hA, hB=hB):
                        nc.tensor.matmul(
                            oaccA[:],
                            vaug[jc][:, (D + 1) * hA:(D + 1) * (hA + 1)],
                            exsA[jc][:],
                            start=(jc == 0), stop=(jc == NJ - 1))
                        nc.tensor.matmul(
                            oaccB[:],
                            vaug[jc][:, (D + 1) * hB:(D + 1) * (hB + 1)],
                            exsB[jc][:],
                            start=(jc == 0), stop=(jc == NJ - 1))

                    for jc in range(NJ):
                        scA = ps.tile([128, CW], F32, name=f"sc{ih}_{hA}_{jc}",
                                      tag="pA", bufs=2)
                        scB = ps.tile([128, CW], F32, name=f"sc{ih}_{hB}_{jc}",
                                      tag="pB", bufs=2)
                        # concurrent row-tiled K=64 pair
                        nc.tensor.matmul(
                            scA[:],
                            k_p[p][0:64, 128 * jc:128 * (jc + 1)],
                            q_r[p][0:64, i0:i0 + CW],
                            start=True, stop=True)
                        nc.tensor.matmul(
                            scB[:],
                            k_p[p][64:128, 128 * jc:128 * (jc + 1)],
                            q_r[p][64:128, i0:i0 + CW],
                            start=True, stop=True)
                        exA = sb.tile([128, CW], BF16, name=f"ex{ih}_{hA}_{jc}",
                                      tag="exA", bufs=3)
                        nc.scalar.activation(exA[:], scA[:],
                                             mybir.ActivationFunctionType.Exp,
                                             scale=float(SCALE),
                                             bias=eshift[:])
                        exB = sb.tile([128, CW], BF16, name=f"ex{ih}_{hB}_{jc}",
                                      tag="exB", bufs=3)
                        if jc in ACT_BOTH:
                            nc.scalar.activation(
                                exB[:], scB[:],
                                mybir.ActivationFunctionType.Exp,
                                scale=float(SCALE), bias=eshift[:])
                        else:
                            nc.vector.tensor_scalar(
                                exB[:].bitcast(I16), scB[:],
                                float(A_SCH), float(B_SCH),
                                mybir.AluOpType.mult, mybir.AluOpType.add)
                        exsA.append(exA)
                        exsB.append(exB)
                        # previous pair's deferred normalization rides inside
                        # this pair's pipeline so the exp stream never pauses
                        if jc == 1 and pending:
                            finalize_head(*pending.pop(0))
                        if jc == 2 and pending:
                            finalize_head(*pending.pop(0))
                        if jc >= 1:
                            emit_o(jc - 1)
                    emit_o(NJ - 1)
                    if p == 0:
                        pending = [(ih, 0, hA, oaccA, CW),
                                   (ih, 0, hB, oaccB, CW)]
                    else:
                        finalize_head(ih, 1, hA, oaccA, CW)
                        finalize_head(ih, 1, hB, oaccB, CW)

                # ---- out-projection partial + RS quarters for this chunk ----
                # p=0 accumulation first: o_pair[0] finalized a pair ago, so
                # these 8 matmuls run while DVE finishes pair-1's norm
                rs_in = dram.tile([C, CW], F16, name=f"rsin{ih}",
                                  tag=f"rsin{ih}")
                pps = []
                for mc in range(8):
                    pp = ps.tile([128, CW], F32, name=f"pp{ih}_{mc}",
                                 tag=PTAGS[mc % 4], bufs=2)
                    nc.tensor.matmul(pp[:],
                                     wproj_sb[0][:, 128 * mc:128 * (mc + 1)],
                                     o_pair[0][:],
                                     start=True, stop=False)
                    pps.append(pp)
                # RS in channel quarters [256, CW]: each starts as soon as its
                # two mc blocks are projected, so the exposed tail is only the
                # last ~256KB collective. Output lands in DRAM as fp16 and is
                # DMA'd straight into `out` (rank r owns rows 64r:64r+64 of
                # each quarter); the f32 upcast happens on the host.
                is_last = ih == NCH - 1
                deng = nc.sync if is_last else nc.gpsimd
                for q in range(4):
                    for mc in (2 * q, 2 * q + 1):
                        nc.tensor.matmul(pps[mc][:],
                                         wproj_sb[1][:, 128 * mc:128 * (mc + 1)],
                                         o_pair[1][:],
                                         start=False, stop=True)
                        po = sb.tile([128, CW], F16, name=f"po{ih}_{mc}",
                                     tag="po", bufs=4)
                        # bias pre-folded into rank-0 cores' partials
                        # (beff input is zeros on other ranks)
                        if mc % 2 == 0:
                            nc.vector.tensor_scalar_add(po[:], pps[mc][:],
                                                        beff_sb[mc][:])
                        else:
                            nc.scalar.activation(
                                po[:], pps[mc][:],
                                mybir.ActivationFunctionType.Identity,
                                bias=beff_sb[mc][:])
                        nc.sync.dma_start(
                            rs_in[128 * mc:128 * (mc + 1), :], po[:])
                    rs_out_q = dram.tile([256 // group_size, CW], F16,
                                         name=f"rsout{ih}_{q}",
                                         tag=f"rsout{ih}_{q}")
                    nc.gpsimd.collective_compute(
                        "ReduceScatter", mybir.AluOpType.add,
                        replica_groups=groups,
                        ins=[rs_in[256 * q:256 * (q + 1), :]],
                        outs=[rs_out_q[:]])
                    deng.dma_start(
                        out.ap()[64 * q:64 * (q + 1), i0:i0 + CW],
                        rs_out_q[:])

    nc.compile()
    return nc


def shard_inputs(x, rope, w_qkv, b_qkv, w_proj, b_proj,
                 n_cores=N_CORES, group_size=4):
    """Per-core input maps. Host-side transposes/casts are part of sharding."""
    # fold the v-bias through the projection into an effective output bias
    b_v = b_qkv[2 * C:3 * C]
    b_eff = (b_proj + b_v @ w_proj.T).astype(np.float32)   # [C]

    in_maps = []
    for c in range(n_cores):
        b = (c // group_size) % B
        g = c % group_size
        heads = range(HL * g, HL * g + HL)

        xTb = np.ascontiguousarray(x[b].T).astype(BF)            # [C, N]

        cosT = rope[b].T[:D, :]                                   # [64, N]
        sinT = rope[b].T[D:, :]
        cos2 = np.vstack([cosT, cosT]).astype(BF)                 # [128, N]
        sgn = np.where(np.arange(128) % 2 == 0, -1.0, 1.0)[:, None]
        sin2s = (np.vstack([sinT, sinT]) * sgn).astype(BF)        # [128, N]

        # qk weight rows ordered [q_h0..q_h3, k_h0..k_h3]
        qk_rows = []
        bqk_rows = []
        for h in heads:
            qk_rows.append(w_qkv[D * h:D * (h + 1), :])           # q rows
            bqk_rows.append(b_qkv[D * h:D * (h + 1)])
        for h in heads:
            qk_rows.append(w_qkv[C + D * h:C + D * (h + 1), :])   # k rows
            bqk_rows.append(b_qkv[C + D * h:C + D * (h + 1)])
        wqk = np.vstack(qk_rows)                                  # [512, C]
        wqkT = np.ascontiguousarray(wqk.T).astype(BF)             # [C, 512]
        bqk_v = np.concatenate(bqk_rows).astype(np.float32)[:, None]

        h0 = HL * g
        wv = w_qkv[2 * C + D * h0:2 * C + D * h0 + CL, :]          # [256, C]
        wvT = np.ascontiguousarray(wv.T).astype(BF)                # [C, 256]

        wp = w_proj[:, D * h0:D * h0 + CL]                         # [C, 256]
        wprojT = np.ascontiguousarray(wp.T).astype(BF)             # [256, C]

        # bias enters via rank 0's pre-RS partials; zeros elsewhere
        if c % group_size == 0:
            beff_full = b_eff.reshape(C, 1).astype(np.float32)
        else:
            beff_full = np.zeros((C, 1), np.float32)

        in_maps.append({
            "xT": xTb, "cos2": cos2, "sin2s": sin2s,
            "wqkT": wqkT, "bqk": bqk_v, "wvT": wvT,
            "wprojT": wprojT, "beff": beff_full,
        })
    return in_maps


def assemble(results, n_cores=N_CORES, group_size=4):
    # out rows [64q:64q+64] on rank r hold channels [256q + 64r, +64)
    out = np.empty((B, N, C), dtype=np.float32)
    for c in range(n_cores):
        b = (c // group_size) % B
        r = c % group_size
        outT_shard = results[c]["out"].astype(np.float32)   # [256, N] f16
        for q in range(4):
            out[b, :, 256 * q + 64 * r:256 * q + 64 * r + 64] = \
                outT_shard[64 * q:64 * (q + 1)].T
    return out


_NC_CACHE = {}


def _get_nc():
    if "nc" not in _NC_CACHE:
        _NC_CACHE["nc"] = build_kernel()
    return _NC_CACHE["nc"]


def _run(inputs, trace=False, tmpdir=None):
    nc = _get_nc()
    inputs = {k: np.asarray(v) for k, v in inputs.items()}
    in_maps = shard_inputs(**inputs)
    res = run_bass_kernel_spmd(nc, in_maps, core_ids=list(range(N_CORES)),
                               trace=trace, tmpdir=tmpdir)
    return assemble(res.results), res


def kernel(**inputs):
    out, _ = _run(inputs)
    return out
